# revision 6
# baseline (speedup 1.0000x reference)
"""DirectedLowRankEdgeScorer TRN2 Bass kernel (8 NeuronCores, SPMD), v2.

logits[b,l,e] = sum_r a[b,I[e],r] * gamma[l,r] * b[b,J[e],r]
  a = relu(H@W1s+b1s)@W2s+b2s,  b = relu(H@W1d+b1d)@W2d+b2d,  H = X[:,-1]

v2 plan (vs v1's 2-sided dma_gather): edges are sharded by J-range so each
core's J-side values come from its OWN node shard.  Per 1024-edge tile:
  - one 1024-idx dma_gather fetches fp16 a+b records for the I side
    (halves the SWDGE descriptor count, the measured bottleneck),
  - the J side is expanded from a 128-node window of the local shard via a
    staircase matmul: S[u,e] = (e >= lo_u) built with one DVE is_ge, times
    telescoped b-differences d[u] = b[u]-b[u-1], so b_exp[:,e] = b[u(e)],
  - a-part is PE-transposed to v-major, DVE product, block-diag gamma matmul.
MLP runs on the node shard in fp16 (fp32 psum) and ends in two half-shard
AllGathers of the fp16 record table, exactly like v1.
"""

import sys
import types

import numpy as np
import ml_dtypes

import bass_rust
import concourse.bass as bass
import concourse.bacc as bacc
import concourse.mybir as mybir
from concourse.bass_utils import run_bass_kernel_spmd
from concourse.tile import TileContext
from concourse.vector_clock import ScopedClock
from concourse.masks import make_identity
from concourse.tile import add_dep_helper

FP16 = np.float16

B, T, N, C = 2, 8, 50000, 64
HID, R, L, E = 128, 16, 12, 1600000
NCORES = 8
NP = 6272                     # nodes per core shard (49*128)
NPAD = NP * NCORES            # 50176 padded node count
H1N, H2N = 3200, 3072         # half-shard split (per-rank rows in rec_h1/rec_h2)
TILE = 1024                   # edges per tile


# ---------------------------------------------------------------- patches
def _patched_drain_and_barrier(self, tick_clock, wait_clock):
    nc = self.nc
    probe = nc.sync.drain()
    wait_clock.add_sem_waits(probe.ins, ScopedClock({None: tick_clock.global_clock}))
    si = probe.ins.sync_info
    waits = list(si.on_wait) if si is not None else []
    if len(waits) > 2:
        si.on_wait.clear()
        si.on_wait.extend(waits[:2])
        for k in range(2, len(waits), 2):
            ni = nc.sync.drain().ins
            ni.sync_info = bass_rust.SyncInfo(on_wait=waits[k:k + 2], on_update=[])
    nc.all_engine_barrier()
    assert self.sems is not None
    popped = nc._tile_sem_poison_stack.pop()
    assert popped is self._sem_poison
    nc.clear_and_free_semaphores(list(self.sems.allocated().values()))
    nc.all_engine_barrier()


TileContext._drain_and_barrier = _patched_drain_and_barrier

if "antenv.axon_hooks" not in sys.modules:
    _mod = types.ModuleType("antenv.axon_hooks")
    _state = {"hook": None}
    _mod.set_axon_ntff_profile_hook = lambda h: _state.__setitem__("hook", h)
    _mod.get_axon_ntff_profile_hook = lambda: _state["hook"]
    sys.modules["antenv.axon_hooks"] = _mod
    try:
        import antenv

        antenv.axon_hooks = _mod
    except Exception:
        pass
    try:
        from trn_agent_boot.trn_boot import _ntff_profile_via_ctypes

        _hook = _ntff_profile_via_ctypes("/opt/axon/libaxon_pjrt.so")
        if _hook is not None:
            _mod.set_axon_ntff_profile_hook(_hook)
    except Exception:
        pass


# ---------------------------------------------------------------- device
_PROGRAM_CACHE = {}


def build_program(nT1, nT2):
    nT = nT1 + nT2
    assert nT % 2 == 0
    nTP = nT // 2
    f32, fp16, i16 = mybir.dt.float32, mybir.dt.float16, mybir.dt.int16

    nc = bacc.Bacc("TRN2", target_bir_lowering=False, num_swdge_queues=4)

    HT = nc.declare_dram_parameter("HT", [B, C, NP], fp16, isOutput=False)
    W1 = nc.declare_dram_parameter("W1", [2, C, HID], fp16, isOutput=False)
    B1 = nc.declare_dram_parameter("B1", [2, HID, 1], f32, isOutput=False)
    W2 = nc.declare_dram_parameter("W2", [2, HID, R], fp16, isOutput=False)
    B2 = nc.declare_dram_parameter("B2", [2, 128, R], f32, isOutput=False)
    GBD = nc.declare_dram_parameter("GBD", [128, 96], fp16, isOutput=False)
    IOTA = nc.declare_dram_parameter("IOTA", [128, TILE], fp16, isOutput=False)
    IDXA = nc.declare_dram_parameter("IDXA", [nT, 128, 64], i16, isOutput=False)
    IDXW = nc.declare_dram_parameter("IDXW", [nT, 128, 8], i16, isOutput=False)
    IDXW2 = nc.declare_dram_parameter("IDXW2", [nT, 128, 8], i16, isOutput=False)
    THL = nc.declare_dram_parameter("THL", [128, nT], f32, isOutput=False)
    OUT = nc.declare_dram_parameter("OUT", [96, nTP * 512], f32, isOutput=True)

    rec_shard = nc.dram_tensor("rec_shard", [NP + 16, 128], fp16)
    rec_h1 = nc.dram_tensor("rec_h1", [NCORES * H1N, 128], fp16, addr_space="Shared")
    rec_h2 = nc.dram_tensor("rec_h2", [NCORES * H2N, 128], fp16, addr_space="Shared")

    with TileContext(nc) as tc:
        with (
            tc.tile_pool(name="const", bufs=1) as constp,
            tc.tile_pool(name="h1p", bufs=1) as h1p,
            tc.tile_pool(name="recp", bufs=3) as recp,
            tc.tile_pool(name="gp", bufs=6) as gp,
            tc.tile_pool(name="bwp", bufs=4) as bwp,
            tc.tile_pool(name="sp", bufs=4) as sp,
            tc.tile_pool(name="dp", bufs=4) as dp,
            tc.tile_pool(name="atp", bufs=4) as atp,
            tc.tile_pool(name="actp", bufs=4) as actp,
            tc.tile_pool(name="prp", bufs=4) as prp,
            tc.tile_pool(name="outp", bufs=3) as outp,
        ):
            w1_s = constp.tile([C, 2, HID], fp16)
            nc.sync.dma_start(w1_s[:], W1[:].rearrange("t c h -> c t h"))
            b1_s = constp.tile([HID, 2, 1], f32)
            nc.sync.dma_start(b1_s[:], B1[:].rearrange("t h o -> h t o"))
            w2_s = constp.tile([HID, 2, R], fp16)
            nc.sync.dma_start(w2_s[:], W2[:].rearrange("t h r -> h t r"))
            b2_s = constp.tile([128, 2, R], f32)
            nc.sync.dma_start(b2_s[:], B2[:].rearrange("t p r -> p t r"))
            ht_s = constp.tile([C, B, NP], fp16)
            nc.sync.dma_start(ht_s[:], HT[:].rearrange("b c n -> c b n"))
            gbd_s = constp.tile([128, 96], fp16)
            nc.sync.dma_start(gbd_s[:], GBD[:])
            iota_s = constp.tile([128, TILE], fp16)
            nc.sync.dma_start(iota_s[:], IOTA[:])
            idxa_all = constp.tile([128, nT, 64], i16)
            nc.sync.dma_start(idxa_all[:], IDXA[:].rearrange("t p x -> p t x"))
            idxw_all = constp.tile([128, nT, 8], i16)
            nc.sync.dma_start(idxw_all[:], IDXW[:].rearrange("t p x -> p t x"))
            idxw2_all = constp.tile([128, nT, 8], i16)
            nc.sync.dma_start(idxw2_all[:], IDXW2[:].rearrange("t p x -> p t x"))
            zp = constp.tile([16, 128], fp16)
            nc.vector.memset(zp[:], 0.0)
            zdma = nc.sync.dma_start(rec_shard[NP:NP + 16, :], zp[:])
            thl_s = constp.tile([128, nT], f32)
            nc.sync.dma_start(thl_s[:], THL[:])
            ident = constp.tile([128, 128], fp16)
            make_identity(nc, ident[:])

            # ---- MLP passes; each pass ends with its half-shard AllGather
            cc_insts = []
            last_rec_dma = None
            mlp_ps = tc.tile_pool(name="psX", bufs=2, space="PSUM")
            psX = mlp_ps.__enter__()
            mlp_ps2 = tc.tile_pool(name="ps2", bufs=2, space="PSUM")
            ps2 = mlp_ps2.__enter__()
            for (p0, psz) in ((0, H1N), (H1N, H2N)):
                h1t = {}
                for t in range(2):
                    for b in range(B):
                        h1x = h1p.tile([HID, max(H1N, H2N)], fp16, tag=f"h1_{t}_{b}")
                        h1t[(t, b)] = h1x
                for n0 in range(0, psz, 512):
                    csz = min(512, psz - n0)
                    for t in range(2):
                        for b in range(B):
                            p1 = psX.tile([HID, 512], f32, tag="px")
                            nc.tensor.matmul(
                                p1[:, :csz],
                                w1_s[:, t, :],
                                ht_s[:, b, p0 + n0:p0 + n0 + csz],
                            )
                            nc.scalar.activation(
                                h1t[(t, b)][:, n0:n0 + csz], p1[:, :csz],
                                mybir.ActivationFunctionType.Relu,
                                bias=b1_s[:, t, :], scale=1.0,
                            )
                rec_dmas = []
                for s in range(psz // 128):
                    rec = recp.tile([128, 64], fp16, tag="rec")
                    for t in range(2):
                        for b in range(B):
                            p2 = ps2.tile([128, R], f32, tag="p2")
                            nc.tensor.matmul(
                                p2[:],
                                h1t[(t, b)][:, s * 128:(s + 1) * 128],
                                w2_s[:, t, :],
                            )
                            co = 32 * t + 16 * b
                            nc.vector.tensor_add(
                                rec[:, co:co + 16], p2[:], b2_s[:, t, :]
                            )
                    n0 = p0 + s * 128
                    di = nc.sync.dma_start(rec_shard[n0:n0 + 128, 0:64], rec[:])
                    rec_dmas.append(di)
                dst = rec_h1 if p0 == 0 else rec_h2
                cc = nc.gpsimd.collective_compute(
                    "AllGather",
                    mybir.AluOpType.bypass,
                    replica_groups=[list(range(NCORES))],
                    ins=[rec_shard[p0:p0 + psz, :]],
                    outs=[dst[:]],
                )
                for di in rec_dmas:
                    add_dep_helper(cc.ins, di.ins, True, "cc waits rec dmas")
                if cc_insts:
                    add_dep_helper(cc.ins, cc_insts[-1].ins, True, "cc order")
                cc_insts.append(cc)
                last_rec_dma = rec_dmas[-1]

            mlp_ps2.__exit__(None, None, None)
            mlp_ps.__exit__(None, None, None)

            # ---- score tiles
            score_ps = [tc.tile_pool(name="psB", bufs=2, space="PSUM"),
                        tc.tile_pool(name="psT", bufs=2, space="PSUM"),
                        tc.tile_pool(name="psL", bufs=2, space="PSUM")]
            psB, psT, psL = [p.__enter__() for p in score_ps]
            pL = None
            for t in range(nT):
                g = 0 if t < nT1 else 1
                recA = rec_h1 if g == 0 else rec_h2
                ccA = cc_insts[g]

                gA = gp.tile([128, 8, 128], fp16, tag="gA")
                ga_i = nc.gpsimd.dma_gather(
                    gA[:], recA[:], idxa_all[:, t, 0:64],
                    num_idxs=TILE, num_idxs_reg=TILE, elem_size=128,
                    single_packet=False, queue_num=t % 4,
                )
                add_dep_helper(ga_i.ins, ccA.ins, True, "gather waits cc")

                bw = bwp.tile([128, 1, 128], fp16, tag="bw")
                bw_i = nc.gpsimd.dma_gather(
                    bw[:], rec_shard[:], idxw_all[:, t, 0:8],
                    num_idxs=128, num_idxs_reg=128, elem_size=128,
                    single_packet=False, queue_num=t % 4,
                )
                add_dep_helper(bw_i.ins, last_rec_dma.ins, True, "bw waits mlp")
                bw2 = bwp.tile([128, 1, 128], fp16, tag="bw2")
                bw2_i = nc.gpsimd.dma_gather(
                    bw2[:], rec_shard[:], idxw2_all[:, t, 0:8],
                    num_idxs=128, num_idxs_reg=128, elem_size=128,
                    single_packet=False, queue_num=t % 4,
                )
                add_dep_helper(bw2_i.ins, last_rec_dma.ins, True, "bw2 waits mlp")
                add_dep_helper(bw2_i.ins, zdma.ins, True, "bw2 waits zero row")

                S = sp.tile([128, TILE], fp16, tag="S")
                nc.vector.tensor_scalar(
                    S[:], iota_s[:], thl_s[:, t:t + 1], None,
                    mybir.AluOpType.is_ge,
                )
                d = dp.tile([128, 32], fp16, tag="d")
                nc.vector.tensor_sub(d[:], bw[:, 0, 32:64], bw2[:, 0, 32:64])

                aC = actp.tile([128, 8, 32], fp16, tag="aC")
                nc.scalar.copy(aC[:], gA[:, :, 0:32])
                prod = prp.tile([128, 256], fp16, tag="prod")
                for c2 in range(2):
                    bexp = psB.tile([32, 512], f32, tag="bexp")
                    nc.tensor.matmul(bexp[:], d[:], S[:, 512 * c2:512 * c2 + 512])
                    pT = psT.tile([128, 128], fp16, tag="pT")
                    nc.tensor.transpose(pT[:], aC[:, 4 * c2:4 * c2 + 4, :], ident[:])
                    aT = atp.tile([128, 128], fp16, tag="aT")
                    nc.scalar.copy(aT[:], pT[:])
                    for g4 in range(4):
                        nc.vector.tensor_mul(
                            prod[32 * g4:32 * g4 + 32, 128 * c2:128 * c2 + 128],
                            aT[32 * g4:32 * g4 + 32, :],
                            bexp[0:32, 128 * g4:128 * g4 + 128],
                        )

                j = t % 2
                if j == 0:
                    pL = psL.tile([96, 512], f32, tag="pL")
                nc.tensor.matmul(pL[:, 256 * j:256 * j + 256], gbd_s[:], prod[:])
                if j == 1:
                    P = t // 2
                    outS = outp.tile([96, 512], f32, tag="outS")
                    nc.scalar.copy(outS[:], pL[:])
                    nc.sync.dma_start(OUT[:, 512 * P:512 * (P + 1)], outS[:])
            for p in reversed(score_ps):
                p.__exit__(None, None, None)

    nc.finalize()
    return nc


# ---------------------------------------------------------------- host
def _wrap_idx(flat_idx, kg):
    """[kg] int16 -> [128, kg//16] wrapped-16, replicated x8."""
    w = flat_idx.reshape(kg // 16, 16).T
    return np.tile(w, (8, 1))


def _rowbuf(nodes):
    r = nodes // NP
    i = nodes % NP
    in1 = i < H1N
    row = np.where(in1, H1N * r + i, H2N * r + (i - H1N))
    return row, in1


def kernel(X, edge_index, W1s, b1s, W2s, b2s, W1d, b1d, W2d, b2d, gamma):
    X = np.asarray(X)
    edge_index = np.asarray(edge_index)
    H = np.ascontiguousarray(X[:, -1]).astype(np.float32)          # (B, N, C)
    Hp = np.zeros((B, NPAD, C), np.float32)
    Hp[:, :N] = H

    I = edge_index[0].astype(np.int64)
    J = edge_index[1].astype(np.int64)
    core = J // NP

    percore = []
    nT1s, nT2s = [], []
    for c in range(NCORES):
        sel = np.where(core == c)[0]
        rI, b1I = _rowbuf(I[sel])
        Jloc = J[sel] - c * NP
        grp = np.where(b1I, 0, 1)
        order = np.lexsort((Jloc, grp))
        percore.append((sel[order], rI[order], Jloc[order], grp[order]))
        nT1s.append(int((grp == 0).sum()))
        nT2s.append(int((grp == 1).sum()))
    nT1 = -(-max(nT1s) // TILE)
    nT2 = -(-max(nT2s) // TILE)
    if (nT1 + nT2) % 2:
        nT2 += 1
    nT = nT1 + nT2

    key = (nT1, nT2)
    if key not in _PROGRAM_CACHE:
        _PROGRAM_CACHE[key] = build_program(nT1, nT2)
    nc = _PROGRAM_CACHE[key]

    # shared weight tensors
    W1 = np.stack([W1s, W1d]).astype(FP16)                        # (2, C, HID)
    B1 = np.stack([b1s, b1d]).astype(np.float32)[:, :, None]      # (2, HID, 1)
    W2 = np.stack([W2s, W2d]).astype(FP16)                        # (2, HID, R)
    B2 = np.stack(
        [np.tile(b2s[None, :], (128, 1)), np.tile(b2d[None, :], (128, 1))]
    ).astype(np.float32)                                          # (2, 128, R)

    gbd = np.zeros((128, 96), np.float32)
    gT = np.asarray(gamma, np.float32).T                          # (R, L)
    for g in range(4):
        for b in range(B):
            gbd[32 * g + 16 * b:32 * g + 16 * b + 16,
                24 * g + 12 * b:24 * g + 12 * b + 12] = gT
    GBDh = gbd.astype(FP16)

    iota = np.tile(np.arange(TILE, dtype=np.float64), (128, 1)).astype(FP16)

    in_maps = []
    unperm = []
    for c in range(NCORES):
        sel_s, rI_s, Jl_s, grp_s = percore[c]
        cnt1 = int((grp_s == 0).sum())
        cnt2 = len(sel_s) - cnt1
        idxA = np.zeros((nT, 128, 64), np.int16)
        idxW = np.zeros((nT, 128, 8), np.int16)
        idxW2 = np.zeros((nT, 128, 8), np.int16)
        thl = np.full((nT, 128), TILE, np.float32)
        pad_pos = np.full(nT * TILE, -1, np.int64)
        for gidx, base_t, cnt, off in ((0, 0, cnt1, 0), (1, nT1, cnt2, cnt1)):
            ntile_g = nT1 if gidx == 0 else nT2
            rI_g = rI_s[off:off + cnt]
            Jl_g = Jl_s[off:off + cnt]
            sel_g = sel_s[off:off + cnt]
            for t in range(ntile_g):
                e0 = t * TILE
                e1 = min(e0 + TILE, cnt)
                k = e1 - e0
                tt = base_t + t
                if k <= 0:
                    idxW[tt] = _wrap_idx(np.arange(128, dtype=np.int16), 128)
                    idxW2[tt] = _wrap_idx(np.arange(128, dtype=np.int16), 128)
                    continue
                ji = Jl_g[e0:e1]
                w0 = int(min(ji.min(), NP - 128))
                u = ji - w0
                assert u.max() < 128, (c, tt, int(u.max()))
                upad = np.concatenate([u, np.full(TILE - k, 127, np.int64)])
                ia = np.zeros(TILE, np.int64)
                ia[:k] = rI_g[e0:e1]
                idxA[tt] = _wrap_idx(ia.astype(np.int16), TILE)
                idxW[tt] = _wrap_idx(
                    (w0 + np.arange(128)).astype(np.int16), 128)
                w2 = w0 - 1 + np.arange(128)
                w2[0] = NP
                idxW2[tt] = _wrap_idx(w2.astype(np.int16), 128)
                thl[tt] = np.searchsorted(upad, np.arange(128), "left")
                pad_pos[tt * TILE: tt * TILE + k] = sel_g[e0:e1]
        unperm.append(pad_pos)

        HTs = np.ascontiguousarray(
            Hp[:, c * NP:(c + 1) * NP, :].transpose(0, 2, 1)
        ).astype(FP16)                                            # (B, C, NP)
        in_maps.append({
            "HT": HTs, "W1": W1, "B1": B1, "W2": W2, "B2": B2,
            "GBD": GBDh, "IOTA": iota, "IDXA": idxA, "IDXW": idxW,
            "IDXW2": idxW2,
            "THL": np.ascontiguousarray(thl.T),
        })

    import os
    import tempfile
    trace = bool(os.environ.get("BASS_KERNEL_TRACE"))
    tdir = None
    if trace:
        base = "/root/problem/work"
        tdir = tempfile.mkdtemp(prefix="ktrace_", dir=base if os.path.isdir(base) else None)
    res = run_bass_kernel_spmd(
        nc, in_maps, list(range(NCORES)), trace=trace, tmpdir=tdir,
    )
    if trace:
        kernel.last_trace_dir = tdir
        kernel.last_exec_time_ns = res.exec_time_ns

    logits = np.empty((B, L, E), np.float32)
    for c in range(NCORES):
        dev = res.results[c]["OUT"]                               # (96, nTP*512)
        # psum row p = 24*g4 + 12*b + l ; col = 512*P + 256*j + 128*c2 + e
        # edge slot = (2P+j)*1024 + 512*c2 + 128*g4 + e
        dv = np.asarray(dev).reshape(4, 2, L, nT // 2, 2, 2, 128)
        dv = dv.transpose(1, 2, 3, 4, 5, 0, 6).reshape(B, L, nT * TILE)
        pad_pos = unperm[c]
        valid = pad_pos >= 0
        logits[:, :, pad_pos[valid]] = dv[:, :, valid]
    return logits


# revision 8
# speedup vs baseline: 1.1133x; 1.1133x over previous
"""DirectedLowRankEdgeScorer TRN2 Bass kernel (8 NeuronCores, SPMD), v2.

logits[b,l,e] = sum_r a[b,I[e],r] * gamma[l,r] * b[b,J[e],r]
  a = relu(H@W1s+b1s)@W2s+b2s,  b = relu(H@W1d+b1d)@W2d+b2d,  H = X[:,-1]

v2 plan (vs v1's 2-sided dma_gather): edges are sharded by J-range so each
core's J-side values come from its OWN node shard.  Per 1024-edge tile:
  - one 1024-idx dma_gather fetches fp16 a+b records for the I side
    (halves the SWDGE descriptor count, the measured bottleneck),
  - the J side is expanded from a 128-node window of the local shard via a
    staircase matmul: S[u,e] = (e >= lo_u) built with one DVE is_ge, times
    telescoped b-differences d[u] = b[u]-b[u-1], so b_exp[:,e] = b[u(e)],
  - a-part is PE-transposed to v-major, DVE product, block-diag gamma matmul.
MLP runs on the node shard in fp16 (fp32 psum) and ends in two half-shard
AllGathers of the fp16 record table, exactly like v1.
"""

import sys
import types

import numpy as np
import ml_dtypes

import bass_rust
import concourse.bass as bass
import concourse.bacc as bacc
import concourse.mybir as mybir
from concourse.bass_utils import run_bass_kernel_spmd
from concourse.tile import TileContext
from concourse.vector_clock import ScopedClock
from concourse.masks import make_identity
from concourse.tile import add_dep_helper

FP16 = np.float16

B, T, N, C = 2, 8, 50000, 64
HID, R, L, E = 128, 16, 12, 1600000
NCORES = 8
NP = 6272                     # nodes per core shard (49*128)
NPAD = NP * NCORES            # 50176 padded node count
H1N, H2N = 3200, 3072         # half-shard split (per-rank rows in rec_h1/rec_h2)
TILE = 1024                   # edges per tile


# ---------------------------------------------------------------- patches
def _patched_drain_and_barrier(self, tick_clock, wait_clock):
    nc = self.nc
    probe = nc.sync.drain()
    wait_clock.add_sem_waits(probe.ins, ScopedClock({None: tick_clock.global_clock}))
    si = probe.ins.sync_info
    waits = list(si.on_wait) if si is not None else []
    if len(waits) > 2:
        si.on_wait.clear()
        si.on_wait.extend(waits[:2])
        for k in range(2, len(waits), 2):
            ni = nc.sync.drain().ins
            ni.sync_info = bass_rust.SyncInfo(on_wait=waits[k:k + 2], on_update=[])
    nc.all_engine_barrier()
    assert self.sems is not None
    popped = nc._tile_sem_poison_stack.pop()
    assert popped is self._sem_poison
    nc.clear_and_free_semaphores(list(self.sems.allocated().values()))
    nc.all_engine_barrier()


TileContext._drain_and_barrier = _patched_drain_and_barrier

if "antenv.axon_hooks" not in sys.modules:
    _mod = types.ModuleType("antenv.axon_hooks")
    _state = {"hook": None}
    _mod.set_axon_ntff_profile_hook = lambda h: _state.__setitem__("hook", h)
    _mod.get_axon_ntff_profile_hook = lambda: _state["hook"]
    sys.modules["antenv.axon_hooks"] = _mod
    try:
        import antenv

        antenv.axon_hooks = _mod
    except Exception:
        pass
    try:
        from trn_agent_boot.trn_boot import _ntff_profile_via_ctypes

        _hook = _ntff_profile_via_ctypes("/opt/axon/libaxon_pjrt.so")
        if _hook is not None:
            _mod.set_axon_ntff_profile_hook(_hook)
    except Exception:
        pass


# ---------------------------------------------------------------- device
_PROGRAM_CACHE = {}


def build_program(nT1, nT2):
    nT = nT1 + nT2
    assert nT % 2 == 0
    nTP = nT // 2
    f32, fp16, i16 = mybir.dt.float32, mybir.dt.float16, mybir.dt.int16

    nc = bacc.Bacc("TRN2", target_bir_lowering=False, num_swdge_queues=4)

    HT = nc.declare_dram_parameter("HT", [B, C, NP], fp16, isOutput=False)
    W1 = nc.declare_dram_parameter("W1", [2, C, HID], fp16, isOutput=False)
    B1 = nc.declare_dram_parameter("B1", [2, HID, 1], f32, isOutput=False)
    W2 = nc.declare_dram_parameter("W2", [2, HID, R], fp16, isOutput=False)
    B2 = nc.declare_dram_parameter("B2", [2, 128, R], f32, isOutput=False)
    GBD = nc.declare_dram_parameter("GBD", [128, 96], fp16, isOutput=False)
    IOTA = nc.declare_dram_parameter("IOTA", [128, TILE], fp16, isOutput=False)
    IDXA = nc.declare_dram_parameter("IDXA", [nT, 128, 64], i16, isOutput=False)
    IDXW = nc.declare_dram_parameter("IDXW", [nT, 128, 8], i16, isOutput=False)
    IDXW2 = nc.declare_dram_parameter("IDXW2", [nT, 128, 8], i16, isOutput=False)
    THL = nc.declare_dram_parameter("THL", [128, nT], f32, isOutput=False)
    OUT = nc.declare_dram_parameter("OUT", [96, nTP * 512], f32, isOutput=True)

    rec_shard = nc.dram_tensor("rec_shard", [NP + 16, 128], fp16)
    rec_h1 = nc.dram_tensor("rec_h1", [NCORES * H1N, 128], fp16, addr_space="Shared")
    rec_h2 = nc.dram_tensor("rec_h2", [NCORES * H2N, 128], fp16, addr_space="Shared")

    with TileContext(nc) as tc:
        with (
            tc.tile_pool(name="const", bufs=1) as constp,
            tc.tile_pool(name="h1p", bufs=1) as h1p,
            tc.tile_pool(name="recp", bufs=3) as recp,
            tc.tile_pool(name="gp", bufs=8) as gp,
            tc.tile_pool(name="bwp", bufs=6) as bwp,
            tc.tile_pool(name="sp", bufs=6) as sp,
            tc.tile_pool(name="dp", bufs=6) as dp,
            tc.tile_pool(name="atp", bufs=6) as atp,
            tc.tile_pool(name="actp", bufs=6) as actp,
            tc.tile_pool(name="prp", bufs=6) as prp,
            tc.tile_pool(name="outp", bufs=4) as outp,
        ):
            w1_s = constp.tile([C, 2, HID], fp16)
            nc.sync.dma_start(w1_s[:], W1[:].rearrange("t c h -> c t h"))
            b1_s = constp.tile([HID, 2, 1], f32)
            nc.sync.dma_start(b1_s[:], B1[:].rearrange("t h o -> h t o"))
            w2_s = constp.tile([HID, 2, R], fp16)
            nc.sync.dma_start(w2_s[:], W2[:].rearrange("t h r -> h t r"))
            b2_s = constp.tile([128, 2, R], f32)
            nc.sync.dma_start(b2_s[:], B2[:].rearrange("t p r -> p t r"))
            ht_s = constp.tile([C, B, NP], fp16)
            nc.sync.dma_start(ht_s[:], HT[:].rearrange("b c n -> c b n"))
            gbd_s = constp.tile([128, 96], fp16)
            nc.sync.dma_start(gbd_s[:], GBD[:])
            iota_s = constp.tile([128, TILE], fp16)
            nc.sync.dma_start(iota_s[:], IOTA[:])
            idxa_all = constp.tile([128, nT, 64], i16)
            nc.sync.dma_start(idxa_all[:], IDXA[:].rearrange("t p x -> p t x"))
            idxw_all = constp.tile([128, nT, 8], i16)
            nc.sync.dma_start(idxw_all[:], IDXW[:].rearrange("t p x -> p t x"))
            idxw2_all = constp.tile([128, nT, 8], i16)
            nc.sync.dma_start(idxw2_all[:], IDXW2[:].rearrange("t p x -> p t x"))
            zp = constp.tile([16, 128], fp16)
            nc.vector.memset(zp[:], 0.0)
            zdma = nc.sync.dma_start(rec_shard[NP:NP + 16, :], zp[:])
            thl_s = constp.tile([128, nT], f32)
            nc.sync.dma_start(thl_s[:], THL[:])
            ident = constp.tile([128, 128], fp16)
            make_identity(nc, ident[:])

            # ---- MLP passes; each pass ends with its half-shard AllGather
            cc_insts = []
            last_rec_dma = None
            mlp_ps = tc.tile_pool(name="psX", bufs=2, space="PSUM")
            psX = mlp_ps.__enter__()
            mlp_ps2 = tc.tile_pool(name="ps2", bufs=2, space="PSUM")
            ps2 = mlp_ps2.__enter__()
            for (p0, psz) in ((0, H1N), (H1N, H2N)):
                h1t = {}
                for t in range(2):
                    for b in range(B):
                        h1x = h1p.tile([HID, max(H1N, H2N)], fp16, tag=f"h1_{t}_{b}")
                        h1t[(t, b)] = h1x
                for n0 in range(0, psz, 512):
                    csz = min(512, psz - n0)
                    for t in range(2):
                        for b in range(B):
                            p1 = psX.tile([HID, 512], f32, tag="px")
                            nc.tensor.matmul(
                                p1[:, :csz],
                                w1_s[:, t, :],
                                ht_s[:, b, p0 + n0:p0 + n0 + csz],
                            )
                            nc.scalar.activation(
                                h1t[(t, b)][:, n0:n0 + csz], p1[:, :csz],
                                mybir.ActivationFunctionType.Relu,
                                bias=b1_s[:, t, :], scale=1.0,
                            )
                rec_dmas = []
                for s in range(psz // 128):
                    rec = recp.tile([128, 64], fp16, tag="rec")
                    for t in range(2):
                        for b in range(B):
                            p2 = ps2.tile([128, R], f32, tag="p2")
                            nc.tensor.matmul(
                                p2[:],
                                h1t[(t, b)][:, s * 128:(s + 1) * 128],
                                w2_s[:, t, :],
                            )
                            co = 32 * t + 16 * b
                            nc.vector.tensor_add(
                                rec[:, co:co + 16], p2[:], b2_s[:, t, :]
                            )
                    n0 = p0 + s * 128
                    di = nc.sync.dma_start(rec_shard[n0:n0 + 128, 0:64], rec[:])
                    rec_dmas.append(di)
                dst = rec_h1 if p0 == 0 else rec_h2
                cc = nc.gpsimd.collective_compute(
                    "AllGather",
                    mybir.AluOpType.bypass,
                    replica_groups=[list(range(NCORES))],
                    ins=[rec_shard[p0:p0 + psz, :]],
                    outs=[dst[:]],
                )
                for di in rec_dmas:
                    add_dep_helper(cc.ins, di.ins, True, "cc waits rec dmas")
                if cc_insts:
                    add_dep_helper(cc.ins, cc_insts[-1].ins, True, "cc order")
                cc_insts.append(cc)
                last_rec_dma = rec_dmas[-1]

            mlp_ps2.__exit__(None, None, None)
            mlp_ps.__exit__(None, None, None)

            # ---- score tiles
            score_ps = [tc.tile_pool(name="psB", bufs=2, space="PSUM"),
                        tc.tile_pool(name="psT", bufs=2, space="PSUM"),
                        tc.tile_pool(name="psL", bufs=2, space="PSUM")]
            psB, psT, psL = [p.__enter__() for p in score_ps]
            pL = None
            for t in range(nT):
                g = 0 if t < nT1 else 1
                recA = rec_h1 if g == 0 else rec_h2
                ccA = cc_insts[g]

                gA = gp.tile([128, 8, 128], fp16, tag="gA")
                ga_i = nc.gpsimd.dma_gather(
                    gA[:], recA[:], idxa_all[:, t, 0:64],
                    num_idxs=TILE, num_idxs_reg=TILE, elem_size=128,
                    single_packet=False, queue_num=t % 4,
                )
                add_dep_helper(ga_i.ins, ccA.ins, True, "gather waits cc")

                bw = bwp.tile([128, 1, 128], fp16, tag="bw")
                bw_i = nc.gpsimd.dma_gather(
                    bw[:], rec_shard[:], idxw_all[:, t, 0:8],
                    num_idxs=128, num_idxs_reg=128, elem_size=128,
                    single_packet=False, queue_num=(t + 1) % 4,
                )
                add_dep_helper(bw_i.ins, last_rec_dma.ins, True, "bw waits mlp")
                bw2 = bwp.tile([128, 1, 128], fp16, tag="bw2")
                bw2_i = nc.gpsimd.dma_gather(
                    bw2[:], rec_shard[:], idxw2_all[:, t, 0:8],
                    num_idxs=128, num_idxs_reg=128, elem_size=128,
                    single_packet=False, queue_num=(t + 2) % 4,
                )
                add_dep_helper(bw2_i.ins, last_rec_dma.ins, True, "bw2 waits mlp")
                add_dep_helper(bw2_i.ins, zdma.ins, True, "bw2 waits zero row")

                S = sp.tile([128, TILE], fp16, tag="S")
                nc.vector.tensor_scalar(
                    S[:], iota_s[:], thl_s[:, t:t + 1], None,
                    mybir.AluOpType.is_ge,
                )
                d4 = dp.tile([128, 4, 32], fp16, tag="d")
                for g4 in range(4):
                    nc.vector.tensor_sub(
                        d4[:, g4, :], bw[:, 0, 32:64], bw2[:, 0, 32:64])

                aC = actp.tile([128, 8, 32], fp16, tag="aC")
                nc.scalar.copy(aC[:], gA[:, :, 0:32])
                bexp = psB.tile([128, 1024], f32, tag="bexp")
                d4f = d4[:].rearrange("p a b -> p (a b)")
                nc.tensor.matmul(bexp[:, 0:512], d4f, S[:, 0:512])
                nc.tensor.matmul(bexp[:, 512:1024], d4f, S[:, 512:1024])
                pT = psT.tile([128, 256], fp16, tag="pT")
                nc.tensor.transpose(pT[:, 0:128], aC[:, 0:4, :], ident[:])
                nc.tensor.transpose(pT[:, 128:256], aC[:, 4:8, :], ident[:])
                aT = atp.tile([128, 256], fp16, tag="aT")
                nc.scalar.copy(aT[:], pT[:])
                prod = prp.tile([128, 256], fp16, tag="prod")
                for g4 in range(4):
                    nc.vector.tensor_mul(
                        prod[32 * g4:32 * g4 + 32, :].rearrange(
                            "p (a b) -> p a b", a=2, b=128),
                        aT[32 * g4:32 * g4 + 32, :].rearrange(
                            "p (a b) -> p a b", a=2, b=128),
                        bexp[32 * g4:32 * g4 + 32, :].rearrange(
                            "p (a b) -> p a b", a=2, b=512)[:, :, 128 * g4:128 * g4 + 128],
                    )

                j = t % 2
                if j == 0:
                    pL = psL.tile([96, 512], f32, tag="pL")
                nc.tensor.matmul(pL[:, 256 * j:256 * j + 256], gbd_s[:], prod[:])
                if j == 1:
                    P = t // 2
                    outS = outp.tile([96, 512], f32, tag="outS")
                    nc.scalar.copy(outS[:], pL[:])
                    nc.sync.dma_start(OUT[:, 512 * P:512 * (P + 1)], outS[:])
            for p in reversed(score_ps):
                p.__exit__(None, None, None)

    nc.finalize()
    return nc


# ---------------------------------------------------------------- host
def _wrap_idx(flat_idx, kg):
    """[kg] int16 -> [128, kg//16] wrapped-16, replicated x8."""
    w = flat_idx.reshape(kg // 16, 16).T
    return np.tile(w, (8, 1))


def _rowbuf(nodes):
    r = nodes // NP
    i = nodes % NP
    in1 = i < H1N
    row = np.where(in1, H1N * r + i, H2N * r + (i - H1N))
    return row, in1


def kernel(X, edge_index, W1s, b1s, W2s, b2s, W1d, b1d, W2d, b2d, gamma):
    X = np.asarray(X)
    edge_index = np.asarray(edge_index)
    H = np.ascontiguousarray(X[:, -1]).astype(np.float32)          # (B, N, C)
    Hp = np.zeros((B, NPAD, C), np.float32)
    Hp[:, :N] = H

    I = edge_index[0].astype(np.int64)
    J = edge_index[1].astype(np.int64)
    core = J // NP

    percore = []
    nT1s, nT2s = [], []
    for c in range(NCORES):
        sel = np.where(core == c)[0]
        rI, b1I = _rowbuf(I[sel])
        Jloc = J[sel] - c * NP
        grp = np.where(b1I, 0, 1)
        order = np.lexsort((Jloc, grp))
        percore.append((sel[order], rI[order], Jloc[order], grp[order]))
        nT1s.append(int((grp == 0).sum()))
        nT2s.append(int((grp == 1).sum()))
    nT1 = -(-max(nT1s) // TILE)
    nT2 = -(-max(nT2s) // TILE)
    if (nT1 + nT2) % 2:
        nT2 += 1
    nT = nT1 + nT2

    key = (nT1, nT2)
    if key not in _PROGRAM_CACHE:
        _PROGRAM_CACHE[key] = build_program(nT1, nT2)
    nc = _PROGRAM_CACHE[key]

    # shared weight tensors
    W1 = np.stack([W1s, W1d]).astype(FP16)                        # (2, C, HID)
    B1 = np.stack([b1s, b1d]).astype(np.float32)[:, :, None]      # (2, HID, 1)
    W2 = np.stack([W2s, W2d]).astype(FP16)                        # (2, HID, R)
    B2 = np.stack(
        [np.tile(b2s[None, :], (128, 1)), np.tile(b2d[None, :], (128, 1))]
    ).astype(np.float32)                                          # (2, 128, R)

    gbd = np.zeros((128, 96), np.float32)
    gT = np.asarray(gamma, np.float32).T                          # (R, L)
    for g in range(4):
        for b in range(B):
            gbd[32 * g + 16 * b:32 * g + 16 * b + 16,
                24 * g + 12 * b:24 * g + 12 * b + 12] = gT
    GBDh = gbd.astype(FP16)

    iota = np.tile(np.arange(TILE, dtype=np.float64), (128, 1)).astype(FP16)

    in_maps = []
    unperm = []
    for c in range(NCORES):
        sel_s, rI_s, Jl_s, grp_s = percore[c]
        cnt1 = int((grp_s == 0).sum())
        cnt2 = len(sel_s) - cnt1
        idxA = np.zeros((nT, 128, 64), np.int16)
        idxW = np.zeros((nT, 128, 8), np.int16)
        idxW2 = np.zeros((nT, 128, 8), np.int16)
        thl = np.full((nT, 128), TILE, np.float32)
        pad_pos = np.full(nT * TILE, -1, np.int64)
        for gidx, base_t, cnt, off in ((0, 0, cnt1, 0), (1, nT1, cnt2, cnt1)):
            ntile_g = nT1 if gidx == 0 else nT2
            rI_g = rI_s[off:off + cnt]
            Jl_g = Jl_s[off:off + cnt]
            sel_g = sel_s[off:off + cnt]
            for t in range(ntile_g):
                e0 = t * TILE
                e1 = min(e0 + TILE, cnt)
                k = e1 - e0
                tt = base_t + t
                if k <= 0:
                    idxW[tt] = _wrap_idx(np.arange(128, dtype=np.int16), 128)
                    idxW2[tt] = _wrap_idx(np.arange(128, dtype=np.int16), 128)
                    continue
                ji = Jl_g[e0:e1]
                w0 = int(min(ji.min(), NP - 128))
                u = ji - w0
                assert u.max() < 128, (c, tt, int(u.max()))
                upad = np.concatenate([u, np.full(TILE - k, 127, np.int64)])
                ia = np.zeros(TILE, np.int64)
                ia[:k] = rI_g[e0:e1]
                idxA[tt] = _wrap_idx(ia.astype(np.int16), TILE)
                idxW[tt] = _wrap_idx(
                    (w0 + np.arange(128)).astype(np.int16), 128)
                w2 = w0 - 1 + np.arange(128)
                w2[0] = NP
                idxW2[tt] = _wrap_idx(w2.astype(np.int16), 128)
                thl[tt] = np.searchsorted(upad, np.arange(128), "left")
                pad_pos[tt * TILE: tt * TILE + k] = sel_g[e0:e1]
        unperm.append(pad_pos)

        HTs = np.ascontiguousarray(
            Hp[:, c * NP:(c + 1) * NP, :].transpose(0, 2, 1)
        ).astype(FP16)                                            # (B, C, NP)
        in_maps.append({
            "HT": HTs, "W1": W1, "B1": B1, "W2": W2, "B2": B2,
            "GBD": GBDh, "IOTA": iota, "IDXA": idxA, "IDXW": idxW,
            "IDXW2": idxW2,
            "THL": np.ascontiguousarray(thl.T),
        })

    import os
    import tempfile
    trace = bool(os.environ.get("BASS_KERNEL_TRACE"))
    tdir = None
    if trace:
        base = "/root/problem/work"
        tdir = tempfile.mkdtemp(prefix="ktrace_", dir=base if os.path.isdir(base) else None)
    res = run_bass_kernel_spmd(
        nc, in_maps, list(range(NCORES)), trace=trace, tmpdir=tdir,
    )
    if trace:
        kernel.last_trace_dir = tdir
        kernel.last_exec_time_ns = res.exec_time_ns

    logits = np.empty((B, L, E), np.float32)
    for c in range(NCORES):
        dev = res.results[c]["OUT"]                               # (96, nTP*512)
        # psum row p = 24*g4 + 12*b + l ; col = 512*P + 256*j + 128*c2 + e
        # edge slot = (2P+j)*1024 + 512*c2 + 128*g4 + e
        dv = np.asarray(dev).reshape(4, 2, L, nT // 2, 2, 2, 128)
        dv = dv.transpose(1, 2, 3, 4, 5, 0, 6).reshape(B, L, nT * TILE)
        pad_pos = unperm[c]
        valid = pad_pos >= 0
        logits[:, :, pad_pos[valid]] = dv[:, :, valid]
    return logits


# revision 10
# speedup vs baseline: 1.2257x; 1.1010x over previous
"""DirectedLowRankEdgeScorer TRN2 Bass kernel (8 NeuronCores, SPMD), v2.

logits[b,l,e] = sum_r a[b,I[e],r] * gamma[l,r] * b[b,J[e],r]
  a = relu(H@W1s+b1s)@W2s+b2s,  b = relu(H@W1d+b1d)@W2d+b2d,  H = X[:,-1]

v2 plan (vs v1's 2-sided dma_gather): edges are sharded by J-range so each
core's J-side values come from its OWN node shard.  Per 1024-edge tile:
  - one 1024-idx dma_gather fetches fp16 a+b records for the I side
    (halves the SWDGE descriptor count, the measured bottleneck),
  - the J side is expanded from a 128-node window of the local shard via a
    staircase matmul: S[u,e] = (e >= lo_u) built with one DVE is_ge, times
    telescoped b-differences d[u] = b[u]-b[u-1], so b_exp[:,e] = b[u(e)],
  - a-part is PE-transposed to v-major, DVE product, block-diag gamma matmul.
MLP runs on the node shard in fp16 (fp32 psum) and ends in two half-shard
AllGathers of the fp16 record table, exactly like v1.
"""

import sys
import types

import numpy as np
import ml_dtypes

import bass_rust
import concourse.bass as bass
import concourse.bacc as bacc
import concourse.mybir as mybir
from concourse.bass_utils import run_bass_kernel_spmd
from concourse.tile import TileContext
from concourse.vector_clock import ScopedClock
from concourse.masks import make_identity
from concourse.tile import add_dep_helper

FP16 = np.float16

B, T, N, C = 2, 8, 50000, 64
HID, R, L, E = 128, 16, 12, 1600000
NCORES = 8
NP = 6272                     # nodes per core shard (49*128)
NPAD = NP * NCORES            # 50176 padded node count
H1N, H2N = 3200, 3072         # half-shard split (per-rank rows in rec_h1/rec_h2)
TILE = 1024                   # edges per tile


# ---------------------------------------------------------------- patches
def _patched_drain_and_barrier(self, tick_clock, wait_clock):
    nc = self.nc
    probe = nc.sync.drain()
    wait_clock.add_sem_waits(probe.ins, ScopedClock({None: tick_clock.global_clock}))
    si = probe.ins.sync_info
    waits = list(si.on_wait) if si is not None else []
    if len(waits) > 2:
        si.on_wait.clear()
        si.on_wait.extend(waits[:2])
        for k in range(2, len(waits), 2):
            ni = nc.sync.drain().ins
            ni.sync_info = bass_rust.SyncInfo(on_wait=waits[k:k + 2], on_update=[])
    nc.all_engine_barrier()
    assert self.sems is not None
    popped = nc._tile_sem_poison_stack.pop()
    assert popped is self._sem_poison
    nc.clear_and_free_semaphores(list(self.sems.allocated().values()))
    nc.all_engine_barrier()


TileContext._drain_and_barrier = _patched_drain_and_barrier

if "antenv.axon_hooks" not in sys.modules:
    _mod = types.ModuleType("antenv.axon_hooks")
    _state = {"hook": None}
    _mod.set_axon_ntff_profile_hook = lambda h: _state.__setitem__("hook", h)
    _mod.get_axon_ntff_profile_hook = lambda: _state["hook"]
    sys.modules["antenv.axon_hooks"] = _mod
    try:
        import antenv

        antenv.axon_hooks = _mod
    except Exception:
        pass
    try:
        from trn_agent_boot.trn_boot import _ntff_profile_via_ctypes

        _hook = _ntff_profile_via_ctypes("/opt/axon/libaxon_pjrt.so")
        if _hook is not None:
            _mod.set_axon_ntff_profile_hook(_hook)
    except Exception:
        pass


# ---------------------------------------------------------------- device
_PROGRAM_CACHE = {}


def build_program(nT1, nT2):
    nT = nT1 + nT2
    assert nT % 2 == 0
    nTP = nT // 2
    f32, fp16, i16 = mybir.dt.float32, mybir.dt.float16, mybir.dt.int16

    nc = bacc.Bacc("TRN2", target_bir_lowering=False, num_swdge_queues=4)

    HT = nc.declare_dram_parameter("HT", [B, C, NP], fp16, isOutput=False)
    W1 = nc.declare_dram_parameter("W1", [2, C, HID], fp16, isOutput=False)
    B1 = nc.declare_dram_parameter("B1", [2, HID, 1], f32, isOutput=False)
    W2 = nc.declare_dram_parameter("W2", [2, HID, R], fp16, isOutput=False)
    B2 = nc.declare_dram_parameter("B2", [2, 128, R], f32, isOutput=False)
    GBD = nc.declare_dram_parameter("GBD", [128, 96], fp16, isOutput=False)
    IOTA = nc.declare_dram_parameter("IOTA", [128, TILE], fp16, isOutput=False)
    IDXA = nc.declare_dram_parameter("IDXA", [nT, 128, 64], i16, isOutput=False)
    IDXW = nc.declare_dram_parameter("IDXW", [nT, 128, 8], i16, isOutput=False)
    IDXW2 = nc.declare_dram_parameter("IDXW2", [nT, 128, 8], i16, isOutput=False)
    THL = nc.declare_dram_parameter("THL", [128, nT], f32, isOutput=False)
    OUT = nc.declare_dram_parameter("OUT", [96, nTP * 512], f32, isOutput=True)

    rec_shard = nc.dram_tensor("rec_shard", [NP + 16, 128], fp16)
    rec_h1 = nc.dram_tensor("rec_h1", [NCORES * H1N, 128], fp16, addr_space="Shared")
    rec_h2 = nc.dram_tensor("rec_h2", [NCORES * H2N, 128], fp16, addr_space="Shared")

    with TileContext(nc) as tc:
        with (
            tc.tile_pool(name="const", bufs=1) as constp,
            tc.tile_pool(name="h1p", bufs=1) as h1p,
            tc.tile_pool(name="recp", bufs=3) as recp,
            tc.tile_pool(name="gp", bufs=8) as gp,
            tc.tile_pool(name="bwp", bufs=6) as bwp,
            tc.tile_pool(name="sp", bufs=6) as sp,
            tc.tile_pool(name="dp", bufs=6) as dp,
            tc.tile_pool(name="atp", bufs=6) as atp,
            tc.tile_pool(name="actp", bufs=6) as actp,
            tc.tile_pool(name="prp", bufs=6) as prp,
            tc.tile_pool(name="outp", bufs=4) as outp,
        ):
            w1_s = constp.tile([C, 2, HID], fp16)
            nc.sync.dma_start(w1_s[:], W1[:].rearrange("t c h -> c t h"))
            b1_s = constp.tile([HID, 2, 1], f32)
            nc.sync.dma_start(b1_s[:], B1[:].rearrange("t h o -> h t o"))
            w2_s = constp.tile([HID, 2, R], fp16)
            nc.sync.dma_start(w2_s[:], W2[:].rearrange("t h r -> h t r"))
            b2_s = constp.tile([128, 2, R], f32)
            nc.sync.dma_start(b2_s[:], B2[:].rearrange("t p r -> p t r"))
            ht_s = constp.tile([C, B, NP], fp16)
            nc.sync.dma_start(ht_s[:], HT[:].rearrange("b c n -> c b n"))
            gbd_s = constp.tile([128, 96], fp16)
            nc.sync.dma_start(gbd_s[:], GBD[:])
            iota_s = constp.tile([128, TILE], fp16)
            nc.sync.dma_start(iota_s[:], IOTA[:])
            idxa_all = constp.tile([128, nT, 64], i16)
            nc.sync.dma_start(idxa_all[:], IDXA[:].rearrange("t p x -> p t x"))
            idxw_all = constp.tile([128, nT, 8], i16)
            nc.sync.dma_start(idxw_all[:], IDXW[:].rearrange("t p x -> p t x"))
            idxw2_all = constp.tile([128, nT, 8], i16)
            nc.sync.dma_start(idxw2_all[:], IDXW2[:].rearrange("t p x -> p t x"))
            zp = constp.tile([16, 128], fp16)
            nc.vector.memset(zp[:], 0.0)
            zdma = nc.sync.dma_start(rec_shard[NP:NP + 16, :], zp[:])
            thl_s = constp.tile([128, nT], f32)
            nc.sync.dma_start(thl_s[:], THL[:])
            ident = constp.tile([128, 128], fp16)
            make_identity(nc, ident[:])

            # ---- MLP passes; each pass ends with its half-shard AllGather
            cc_insts = []
            last_rec_dma = None
            mlp_ps = tc.tile_pool(name="psX", bufs=2, space="PSUM")
            psX = mlp_ps.__enter__()
            mlp_ps2 = tc.tile_pool(name="ps2", bufs=2, space="PSUM")
            ps2 = mlp_ps2.__enter__()
            for (p0, psz) in ((0, H1N), (H1N, H2N)):
                h1t = {}
                for t in range(2):
                    for b in range(B):
                        h1x = h1p.tile([HID, max(H1N, H2N)], fp16, tag=f"h1_{t}_{b}")
                        h1t[(t, b)] = h1x
                for n0 in range(0, psz, 512):
                    csz = min(512, psz - n0)
                    for t in range(2):
                        for b in range(B):
                            p1 = psX.tile([HID, 512], f32, tag="px")
                            nc.tensor.matmul(
                                p1[:, :csz],
                                w1_s[:, t, :],
                                ht_s[:, b, p0 + n0:p0 + n0 + csz],
                            )
                            nc.scalar.activation(
                                h1t[(t, b)][:, n0:n0 + csz], p1[:, :csz],
                                mybir.ActivationFunctionType.Relu,
                                bias=b1_s[:, t, :], scale=1.0,
                            )
                rec_dmas = []
                for s in range(psz // 128):
                    rec = recp.tile([128, 64], fp16, tag="rec")
                    for t in range(2):
                        for b in range(B):
                            p2 = ps2.tile([128, R], f32, tag="p2")
                            nc.tensor.matmul(
                                p2[:],
                                h1t[(t, b)][:, s * 128:(s + 1) * 128],
                                w2_s[:, t, :],
                            )
                            co = 32 * t + 16 * b
                            nc.vector.tensor_add(
                                rec[:, co:co + 16], p2[:], b2_s[:, t, :]
                            )
                    n0 = p0 + s * 128
                    di = nc.sync.dma_start(rec_shard[n0:n0 + 128, 0:64], rec[:])
                    rec_dmas.append(di)
                dst = rec_h1 if p0 == 0 else rec_h2
                cc = nc.gpsimd.collective_compute(
                    "AllGather",
                    mybir.AluOpType.bypass,
                    replica_groups=[list(range(NCORES))],
                    ins=[rec_shard[p0:p0 + psz, :]],
                    outs=[dst[:]],
                )
                for di in rec_dmas:
                    add_dep_helper(cc.ins, di.ins, True, "cc waits rec dmas")
                if cc_insts:
                    add_dep_helper(cc.ins, cc_insts[-1].ins, True, "cc order")
                cc_insts.append(cc)
                last_rec_dma = rec_dmas[-1]

            mlp_ps2.__exit__(None, None, None)
            mlp_ps.__exit__(None, None, None)

            # ---- score tiles
            score_ps = [tc.tile_pool(name="psB", bufs=2, space="PSUM"),
                        tc.tile_pool(name="psT", bufs=2, space="PSUM"),
                        tc.tile_pool(name="psL", bufs=2, space="PSUM")]
            psB, psT, psL = [p.__enter__() for p in score_ps]
            pL = None
            for t in range(nT):
                g = 0 if t < nT1 else 1
                recA = rec_h1 if g == 0 else rec_h2
                ccA = cc_insts[g]

                gA = gp.tile([128, 8, 128], fp16, tag="gA")
                ga_i = nc.gpsimd.dma_gather(
                    gA[:], recA[:], idxa_all[:, t, 0:64],
                    num_idxs=TILE, num_idxs_reg=TILE, elem_size=128,
                    single_packet=False, queue_num=t % 4,
                )
                add_dep_helper(ga_i.ins, ccA.ins, True, "gather waits cc")

                bw = bwp.tile([128, 1, 128], fp16, tag="bw")
                bw_i = nc.gpsimd.dma_gather(
                    bw[:], rec_shard[:], idxw_all[:, t, 0:8],
                    num_idxs=128, num_idxs_reg=128, elem_size=128,
                    single_packet=False, queue_num=(t + 1) % 4,
                )
                add_dep_helper(bw_i.ins, last_rec_dma.ins, True, "bw waits mlp")
                bw2 = bwp.tile([128, 1, 128], fp16, tag="bw2")
                bw2_i = nc.gpsimd.dma_gather(
                    bw2[:], rec_shard[:], idxw2_all[:, t, 0:8],
                    num_idxs=128, num_idxs_reg=128, elem_size=128,
                    single_packet=False, queue_num=(t + 2) % 4,
                )
                add_dep_helper(bw2_i.ins, last_rec_dma.ins, True, "bw2 waits mlp")
                add_dep_helper(bw2_i.ins, zdma.ins, True, "bw2 waits zero row")

                S = sp.tile([128, TILE], fp16, tag="S")
                nc.vector.tensor_scalar(
                    S[:], iota_s[:], thl_s[:, t:t + 1], None,
                    mybir.AluOpType.is_ge,
                )
                d = dp.tile([128, 32], fp16, tag="d")
                nc.vector.tensor_sub(d[:], bw[:, 0, 32:64], bw2[:, 0, 32:64])

                aC = actp.tile([128, 2, 4, 32], fp16, tag="aC")
                nc.scalar.copy(
                    aC[:],
                    gA[:, :, 0:32].rearrange("p (k j) v -> p j k v", k=4, j=2),
                )
                bexpA = psB.tile([64, 256], f32, tag="bexpA")
                bexpB = psB.tile([64, 256], f32, tag="bexpB")
                for k in range(4):
                    dst = bexpA if k < 2 else bexpB
                    nc.tensor.matmul(
                        dst[32 * (k % 2):32 * (k % 2) + 32, :], d[:],
                        S[:, 256 * k:256 * k + 256],
                    )
                pT = psT.tile([128, 256], fp16, tag="pT")
                nc.tensor.transpose(pT[:, 0:128], aC[:, 0, :, :], ident[:])
                nc.tensor.transpose(pT[:, 128:256], aC[:, 1, :, :], ident[:])
                aT = atp.tile([128, 256], fp16, tag="aT")
                nc.scalar.copy(aT[:], pT[:])
                prod = prp.tile([128, 256], fp16, tag="prod")
                nc.vector.tensor_mul(prod[0:64, :], aT[0:64, :], bexpA[:])
                nc.vector.tensor_mul(prod[64:128, :], aT[64:128, :], bexpB[:])

                j = t % 2
                if j == 0:
                    pL = psL.tile([96, 512], f32, tag="pL")
                nc.tensor.matmul(pL[:, 256 * j:256 * j + 256], gbd_s[:], prod[:])
                if j == 1:
                    P = t // 2
                    outS = outp.tile([96, 512], f32, tag="outS")
                    nc.scalar.copy(outS[:], pL[:])
                    nc.sync.dma_start(OUT[:, 512 * P:512 * (P + 1)], outS[:])
            for p in reversed(score_ps):
                p.__exit__(None, None, None)

    nc.finalize()
    return nc


# ---------------------------------------------------------------- host
def _wrap_idx(flat_idx, kg):
    """[kg] int16 -> [128, kg//16] wrapped-16, replicated x8."""
    w = flat_idx.reshape(kg // 16, 16).T
    return np.tile(w, (8, 1))


def _rowbuf(nodes):
    r = nodes // NP
    i = nodes % NP
    in1 = i < H1N
    row = np.where(in1, H1N * r + i, H2N * r + (i - H1N))
    return row, in1


def kernel(X, edge_index, W1s, b1s, W2s, b2s, W1d, b1d, W2d, b2d, gamma):
    X = np.asarray(X)
    edge_index = np.asarray(edge_index)
    H = np.ascontiguousarray(X[:, -1]).astype(np.float32)          # (B, N, C)
    Hp = np.zeros((B, NPAD, C), np.float32)
    Hp[:, :N] = H

    I = edge_index[0].astype(np.int64)
    J = edge_index[1].astype(np.int64)
    core = J // NP

    percore = []
    nT1s, nT2s = [], []
    for c in range(NCORES):
        sel = np.where(core == c)[0]
        rI, b1I = _rowbuf(I[sel])
        Jloc = J[sel] - c * NP
        grp = np.where(b1I, 0, 1)
        order = np.lexsort((Jloc, grp))
        percore.append((sel[order], rI[order], Jloc[order], grp[order]))
        nT1s.append(int((grp == 0).sum()))
        nT2s.append(int((grp == 1).sum()))
    nT1 = -(-max(nT1s) // TILE)
    nT2 = -(-max(nT2s) // TILE)
    if (nT1 + nT2) % 2:
        nT2 += 1
    nT = nT1 + nT2

    key = (nT1, nT2)
    if key not in _PROGRAM_CACHE:
        _PROGRAM_CACHE[key] = build_program(nT1, nT2)
    nc = _PROGRAM_CACHE[key]

    # shared weight tensors
    W1 = np.stack([W1s, W1d]).astype(FP16)                        # (2, C, HID)
    B1 = np.stack([b1s, b1d]).astype(np.float32)[:, :, None]      # (2, HID, 1)
    W2 = np.stack([W2s, W2d]).astype(FP16)                        # (2, HID, R)
    B2 = np.stack(
        [np.tile(b2s[None, :], (128, 1)), np.tile(b2d[None, :], (128, 1))]
    ).astype(np.float32)                                          # (2, 128, R)

    gbd = np.zeros((128, 96), np.float32)
    gT = np.asarray(gamma, np.float32).T                          # (R, L)
    for g in range(4):
        for b in range(B):
            gbd[32 * g + 16 * b:32 * g + 16 * b + 16,
                24 * g + 12 * b:24 * g + 12 * b + 12] = gT
    GBDh = gbd.astype(FP16)

    iota = np.tile(np.arange(TILE, dtype=np.float64), (128, 1)).astype(FP16)

    in_maps = []
    unperm = []
    for c in range(NCORES):
        sel_s, rI_s, Jl_s, grp_s = percore[c]
        cnt1 = int((grp_s == 0).sum())
        cnt2 = len(sel_s) - cnt1
        idxA = np.zeros((nT, 128, 64), np.int16)
        idxW = np.zeros((nT, 128, 8), np.int16)
        idxW2 = np.zeros((nT, 128, 8), np.int16)
        thl = np.full((nT, 128), TILE, np.float32)
        pad_pos = np.full(nT * TILE, -1, np.int64)
        for gidx, base_t, cnt, off in ((0, 0, cnt1, 0), (1, nT1, cnt2, cnt1)):
            ntile_g = nT1 if gidx == 0 else nT2
            rI_g = rI_s[off:off + cnt]
            Jl_g = Jl_s[off:off + cnt]
            sel_g = sel_s[off:off + cnt]
            for t in range(ntile_g):
                e0 = t * TILE
                e1 = min(e0 + TILE, cnt)
                k = e1 - e0
                tt = base_t + t
                if k <= 0:
                    idxW[tt] = _wrap_idx(np.arange(128, dtype=np.int16), 128)
                    idxW2[tt] = _wrap_idx(np.arange(128, dtype=np.int16), 128)
                    continue
                ji = Jl_g[e0:e1]
                w0 = int(min(ji.min(), NP - 128))
                u = ji - w0
                assert u.max() < 128, (c, tt, int(u.max()))
                upad = np.concatenate([u, np.full(TILE - k, 127, np.int64)])
                ia = np.zeros(TILE, np.int64)
                ia[:k] = rI_g[e0:e1]
                idxA[tt] = _wrap_idx(ia.astype(np.int16), TILE)
                idxW[tt] = _wrap_idx(
                    (w0 + np.arange(128)).astype(np.int16), 128)
                w2 = w0 - 1 + np.arange(128)
                w2[0] = NP
                idxW2[tt] = _wrap_idx(w2.astype(np.int16), 128)
                thl[tt] = np.searchsorted(upad, np.arange(128), "left")
                pad_pos[tt * TILE: tt * TILE + k] = sel_g[e0:e1]
        unperm.append(pad_pos)

        HTs = np.ascontiguousarray(
            Hp[:, c * NP:(c + 1) * NP, :].transpose(0, 2, 1)
        ).astype(FP16)                                            # (B, C, NP)
        in_maps.append({
            "HT": HTs, "W1": W1, "B1": B1, "W2": W2, "B2": B2,
            "GBD": GBDh, "IOTA": iota, "IDXA": idxA, "IDXW": idxW,
            "IDXW2": idxW2,
            "THL": np.ascontiguousarray(thl.T),
        })

    import os
    import tempfile
    trace = bool(os.environ.get("BASS_KERNEL_TRACE"))
    tdir = None
    if trace:
        base = "/root/problem/work"
        tdir = tempfile.mkdtemp(prefix="ktrace_", dir=base if os.path.isdir(base) else None)
    res = run_bass_kernel_spmd(
        nc, in_maps, list(range(NCORES)), trace=trace, tmpdir=tdir,
    )
    if trace:
        kernel.last_trace_dir = tdir
        kernel.last_exec_time_ns = res.exec_time_ns

    logits = np.empty((B, L, E), np.float32)
    for c in range(NCORES):
        dev = res.results[c]["OUT"]                               # (96, nTP*512)
        # psum row p = 24*g4 + 12*b + l ; col = 512*P + 256*j + 128*c2 + e
        # edge slot = (2P+j)*1024 + 512*c2 + 128*g4 + e
        dv = np.asarray(dev).reshape(4, 2, L, nT // 2, 2, 256)
        dv = dv.transpose(1, 2, 3, 4, 0, 5).reshape(B, L, nT * TILE)
        pad_pos = unperm[c]
        valid = pad_pos >= 0
        logits[:, :, pad_pos[valid]] = dv[:, :, valid]
    return logits


# revision 12
# speedup vs baseline: 1.2640x; 1.0312x over previous
"""DirectedLowRankEdgeScorer TRN2 Bass kernel (8 NeuronCores, SPMD), v2.

logits[b,l,e] = sum_r a[b,I[e],r] * gamma[l,r] * b[b,J[e],r]
  a = relu(H@W1s+b1s)@W2s+b2s,  b = relu(H@W1d+b1d)@W2d+b2d,  H = X[:,-1]

v2 plan (vs v1's 2-sided dma_gather): edges are sharded by J-range so each
core's J-side values come from its OWN node shard.  Per 1024-edge tile:
  - one 1024-idx dma_gather fetches fp16 a+b records for the I side
    (halves the SWDGE descriptor count, the measured bottleneck),
  - the J side is expanded from a 128-node window of the local shard via a
    staircase matmul: S[u,e] = (e >= lo_u) built with one DVE is_ge, times
    telescoped b-differences d[u] = b[u]-b[u-1], so b_exp[:,e] = b[u(e)],
  - a-part is PE-transposed to v-major, DVE product, block-diag gamma matmul.
MLP runs on the node shard in fp16 (fp32 psum) and ends in two half-shard
AllGathers of the fp16 record table, exactly like v1.
"""

import sys
import types

import numpy as np
import ml_dtypes

import bass_rust
import concourse.bass as bass
import concourse.bacc as bacc
import concourse.mybir as mybir
from concourse.bass_utils import run_bass_kernel_spmd
from concourse.tile import TileContext
from concourse.vector_clock import ScopedClock
from concourse.masks import make_identity
from concourse.tile import add_dep_helper

FP16 = np.float16

B, T, N, C = 2, 8, 50000, 64
HID, R, L, E = 128, 16, 12, 1600000
NCORES = 8
NP = 6272                     # nodes per core shard (49*128)
NPAD = NP * NCORES            # 50176 padded node count
H1N, H2N = 3200, 3072         # half-shard split (per-rank rows in rec_h1/rec_h2)
TILE = 1024                   # edges per tile


# ---------------------------------------------------------------- patches
def _patched_drain_and_barrier(self, tick_clock, wait_clock):
    nc = self.nc
    probe = nc.sync.drain()
    wait_clock.add_sem_waits(probe.ins, ScopedClock({None: tick_clock.global_clock}))
    si = probe.ins.sync_info
    waits = list(si.on_wait) if si is not None else []
    if len(waits) > 2:
        si.on_wait.clear()
        si.on_wait.extend(waits[:2])
        for k in range(2, len(waits), 2):
            ni = nc.sync.drain().ins
            ni.sync_info = bass_rust.SyncInfo(on_wait=waits[k:k + 2], on_update=[])
    nc.all_engine_barrier()
    assert self.sems is not None
    popped = nc._tile_sem_poison_stack.pop()
    assert popped is self._sem_poison
    nc.clear_and_free_semaphores(list(self.sems.allocated().values()))
    nc.all_engine_barrier()


TileContext._drain_and_barrier = _patched_drain_and_barrier

if "antenv.axon_hooks" not in sys.modules:
    _mod = types.ModuleType("antenv.axon_hooks")
    _state = {"hook": None}
    _mod.set_axon_ntff_profile_hook = lambda h: _state.__setitem__("hook", h)
    _mod.get_axon_ntff_profile_hook = lambda: _state["hook"]
    sys.modules["antenv.axon_hooks"] = _mod
    try:
        import antenv

        antenv.axon_hooks = _mod
    except Exception:
        pass
    try:
        from trn_agent_boot.trn_boot import _ntff_profile_via_ctypes

        _hook = _ntff_profile_via_ctypes("/opt/axon/libaxon_pjrt.so")
        if _hook is not None:
            _mod.set_axon_ntff_profile_hook(_hook)
    except Exception:
        pass


# ---------------------------------------------------------------- device
_PROGRAM_CACHE = {}


def build_program(nT1, nT2):
    nT = nT1 + nT2
    assert nT % 2 == 0
    nTP = nT // 2
    f32, fp16, i16 = mybir.dt.float32, mybir.dt.float16, mybir.dt.int16

    nc = bacc.Bacc("TRN2", target_bir_lowering=False, num_swdge_queues=4)

    HT = nc.declare_dram_parameter("HT", [B, C, NP], fp16, isOutput=False)
    W1 = nc.declare_dram_parameter("W1", [2, C, HID], fp16, isOutput=False)
    B1 = nc.declare_dram_parameter("B1", [2, HID, 1], f32, isOutput=False)
    W2 = nc.declare_dram_parameter("W2", [2, HID, R], fp16, isOutput=False)
    B2 = nc.declare_dram_parameter("B2", [2, 128, R], f32, isOutput=False)
    GBD = nc.declare_dram_parameter("GBD", [128, 96], fp16, isOutput=False)
    IOTA = nc.declare_dram_parameter("IOTA", [128, TILE], fp16, isOutput=False)
    IDXA = nc.declare_dram_parameter("IDXA", [nT, 128, 64], i16, isOutput=False)
    IDXW = nc.declare_dram_parameter("IDXW", [nT, 128, 8], i16, isOutput=False)
    IDXW2 = nc.declare_dram_parameter("IDXW2", [nT, 128, 8], i16, isOutput=False)
    THL = nc.declare_dram_parameter("THL", [128, nT], f32, isOutput=False)
    OUT = nc.declare_dram_parameter("OUT", [96, nTP * 512], f32, isOutput=True)

    rec_shard = nc.dram_tensor("rec_shard", [NP + 16, 128], fp16)
    rec_h1 = nc.dram_tensor("rec_h1", [NCORES * H1N, 128], fp16, addr_space="Shared")
    rec_h2 = nc.dram_tensor("rec_h2", [NCORES * H2N, 128], fp16, addr_space="Shared")

    with TileContext(nc) as tc:
        with (
            tc.tile_pool(name="const", bufs=1) as constp,
            tc.tile_pool(name="h1p", bufs=1) as h1p,
            tc.tile_pool(name="recp", bufs=3) as recp,
            tc.tile_pool(name="gp", bufs=8) as gp,
            tc.tile_pool(name="bwp", bufs=6) as bwp,
            tc.tile_pool(name="sp", bufs=6) as sp,
            tc.tile_pool(name="dp", bufs=6) as dp,
            tc.tile_pool(name="atp", bufs=6) as atp,
            tc.tile_pool(name="actp", bufs=6) as actp,
            tc.tile_pool(name="prp", bufs=6) as prp,
            tc.tile_pool(name="outp", bufs=4) as outp,
        ):
            w1_s = constp.tile([C, 2, HID], fp16)
            nc.sync.dma_start(w1_s[:], W1[:].rearrange("t c h -> c t h"))
            b1_s = constp.tile([HID, 2, 1], f32)
            nc.sync.dma_start(b1_s[:], B1[:].rearrange("t h o -> h t o"))
            w2_s = constp.tile([HID, 2, R], fp16)
            nc.sync.dma_start(w2_s[:], W2[:].rearrange("t h r -> h t r"))
            b2_s = constp.tile([128, 2, R], f32)
            nc.sync.dma_start(b2_s[:], B2[:].rearrange("t p r -> p t r"))
            ht_s = constp.tile([C, B, NP], fp16)
            nc.sync.dma_start(ht_s[:], HT[:].rearrange("b c n -> c b n"))
            gbd_s = constp.tile([128, 96], fp16)
            nc.sync.dma_start(gbd_s[:], GBD[:])
            iota_s = constp.tile([128, TILE], fp16)
            nc.sync.dma_start(iota_s[:], IOTA[:])
            idxa_all = constp.tile([128, nT, 64], i16)
            nc.sync.dma_start(idxa_all[:], IDXA[:].rearrange("t p x -> p t x"))
            idxw_all = constp.tile([128, nT, 8], i16)
            nc.sync.dma_start(idxw_all[:], IDXW[:].rearrange("t p x -> p t x"))
            idxw2_all = constp.tile([128, nT, 8], i16)
            nc.sync.dma_start(idxw2_all[:], IDXW2[:].rearrange("t p x -> p t x"))
            zp = constp.tile([16, 128], fp16)
            nc.vector.memset(zp[:], 0.0)
            zdma = nc.sync.dma_start(rec_shard[NP:NP + 16, :], zp[:])
            thl_s = constp.tile([128, nT], f32)
            nc.sync.dma_start(thl_s[:], THL[:])
            ident = constp.tile([128, 128], fp16)
            make_identity(nc, ident[:])

            # ---- MLP passes; each pass ends with its half-shard AllGather
            cc_insts = []
            last_rec_dma = None
            mlp_ps = tc.tile_pool(name="psX", bufs=2, space="PSUM")
            psX = mlp_ps.__enter__()
            mlp_ps2 = tc.tile_pool(name="ps2", bufs=2, space="PSUM")
            ps2 = mlp_ps2.__enter__()
            for (p0, psz) in ((0, H1N), (H1N, H2N)):
                h1t = {}
                for t in range(2):
                    for b in range(B):
                        h1x = h1p.tile([HID, max(H1N, H2N)], fp16, tag=f"h1_{t}_{b}")
                        h1t[(t, b)] = h1x
                for n0 in range(0, psz, 512):
                    csz = min(512, psz - n0)
                    for t in range(2):
                        for b in range(B):
                            p1 = psX.tile([HID, 512], f32, tag="px")
                            nc.tensor.matmul(
                                p1[:, :csz],
                                w1_s[:, t, :],
                                ht_s[:, b, p0 + n0:p0 + n0 + csz],
                            )
                            nc.scalar.activation(
                                h1t[(t, b)][:, n0:n0 + csz], p1[:, :csz],
                                mybir.ActivationFunctionType.Relu,
                                bias=b1_s[:, t, :], scale=1.0,
                            )
                rec_dmas = []
                for s in range(psz // 128):
                    rec = recp.tile([128, 64], fp16, tag="rec")
                    for t in range(2):
                        for b in range(B):
                            p2 = ps2.tile([128, R], f32, tag="p2")
                            nc.tensor.matmul(
                                p2[:],
                                h1t[(t, b)][:, s * 128:(s + 1) * 128],
                                w2_s[:, t, :],
                            )
                            co = 32 * t + 16 * b
                            nc.vector.tensor_add(
                                rec[:, co:co + 16], p2[:], b2_s[:, t, :]
                            )
                    n0 = p0 + s * 128
                    di = nc.sync.dma_start(rec_shard[n0:n0 + 128, 0:64], rec[:])
                    rec_dmas.append(di)
                dst = rec_h1 if p0 == 0 else rec_h2
                cc = nc.gpsimd.collective_compute(
                    "AllGather",
                    mybir.AluOpType.bypass,
                    replica_groups=[list(range(NCORES))],
                    ins=[rec_shard[p0:p0 + psz, :]],
                    outs=[dst[:]],
                )
                for di in rec_dmas:
                    add_dep_helper(cc.ins, di.ins, True, "cc waits rec dmas")
                if cc_insts:
                    add_dep_helper(cc.ins, cc_insts[-1].ins, True, "cc order")
                cc_insts.append(cc)
                last_rec_dma = rec_dmas[-1]

            mlp_ps2.__exit__(None, None, None)
            mlp_ps.__exit__(None, None, None)

            # ---- score tiles
            score_ps = [tc.tile_pool(name="psB", bufs=2, space="PSUM"),
                        tc.tile_pool(name="psT", bufs=2, space="PSUM"),
                        tc.tile_pool(name="psL", bufs=2, space="PSUM")]
            psB, psT, psL = [p.__enter__() for p in score_ps]
            # software-pipelined score loop: each engine works on a
            # different tile to break the cross-engine dependency cycle.
            tiles = {}
            pairs = {}

            def st_gather(t):
                g = 0 if t < nT1 else 1
                recA = rec_h1 if g == 0 else rec_h2
                ccA = cc_insts[g]
                gA = gp.tile([128, 8, 128], fp16, tag="gA")
                ga_i = nc.gpsimd.dma_gather(
                    gA[:], recA[:], idxa_all[:, t, 0:64],
                    num_idxs=TILE, num_idxs_reg=TILE, elem_size=128,
                    single_packet=False, queue_num=t % 4,
                )
                add_dep_helper(ga_i.ins, ccA.ins, True, "gather waits cc")
                bw = bwp.tile([128, 1, 128], fp16, tag="bw")
                bw_i = nc.gpsimd.dma_gather(
                    bw[:], rec_shard[:], idxw_all[:, t, 0:8],
                    num_idxs=128, num_idxs_reg=128, elem_size=128,
                    single_packet=False, queue_num=(t + 1) % 4,
                )
                add_dep_helper(bw_i.ins, last_rec_dma.ins, True, "bw waits mlp")
                bw2 = bwp.tile([128, 1, 128], fp16, tag="bw2")
                bw2_i = nc.gpsimd.dma_gather(
                    bw2[:], rec_shard[:], idxw2_all[:, t, 0:8],
                    num_idxs=128, num_idxs_reg=128, elem_size=128,
                    single_packet=False, queue_num=(t + 2) % 4,
                )
                add_dep_helper(bw2_i.ins, last_rec_dma.ins, True, "bw2 waits mlp")
                add_dep_helper(bw2_i.ins, zdma.ins, True, "bw2 waits zero row")
                tiles[t] = {"gA": gA, "bw": bw, "bw2": bw2}

            def st_sd(t):
                z = tiles[t]
                S = sp.tile([128, TILE], fp16, tag="S")
                nc.vector.tensor_scalar(
                    S[:], iota_s[:], thl_s[:, t:t + 1], None,
                    mybir.AluOpType.is_ge,
                )
                d = dp.tile([128, 32], fp16, tag="d")
                nc.vector.tensor_sub(d[:], z["bw"][:, 0, 32:64], z["bw2"][:, 0, 32:64])
                z["S"], z["d"] = S, d

            def st_ac(t):
                z = tiles[t]
                aC = actp.tile([128, 2, 4, 32], fp16, tag="aC")
                nc.scalar.copy(
                    aC[:],
                    z["gA"][:, :, 0:32].rearrange("p (k j) v -> p j k v", k=4, j=2),
                )
                z["aC"] = aC

            def st_pe(t):
                z = tiles[t]
                bexpA = psB.tile([64, 256], f32, tag="bexpA")
                bexpB = psB.tile([64, 256], f32, tag="bexpB")
                for k in range(4):
                    dst = bexpA if k < 2 else bexpB
                    nc.tensor.matmul(
                        dst[32 * (k % 2):32 * (k % 2) + 32, :], z["d"][:],
                        z["S"][:, 256 * k:256 * k + 256],
                    )
                pT = psT.tile([128, 256], fp16, tag="pT")
                nc.tensor.transpose(pT[:, 0:128], z["aC"][:, 0, :, :], ident[:])
                nc.tensor.transpose(pT[:, 128:256], z["aC"][:, 1, :, :], ident[:])
                z["bexpA"], z["bexpB"], z["pT"] = bexpA, bexpB, pT

            def st_at_mul(t):
                z = tiles[t]
                aT = atp.tile([128, 256], fp16, tag="aT")
                nc.scalar.copy(aT[:], z["pT"][:])
                prod = prp.tile([128, 256], fp16, tag="prod")
                nc.vector.tensor_mul(prod[0:64, :], aT[0:64, :], z["bexpA"][:])
                nc.vector.tensor_mul(prod[64:128, :], aT[64:128, :], z["bexpB"][:])
                z["prod"] = prod

            def st_gbd(t):
                z = tiles[t]
                P, j = t // 2, t % 2
                if j == 0:
                    pLt = psL.tile([96, 512], f32, tag="pL", name="pLt")
                    pairs[P] = pLt
                nc.tensor.matmul(pairs[P][:, 256 * j:256 * j + 256],
                                 gbd_s[:], z["prod"][:])

            def st_out(t):
                P, j = t // 2, t % 2
                if j == 1:
                    outS = outp.tile([96, 512], f32, tag="outS")
                    nc.scalar.copy(outS[:], pairs[P][:])
                    nc.sync.dma_start(OUT[:, 512 * P:512 * (P + 1)], outS[:])
                    del pairs[P]

            def live(t):
                return 0 <= t < nT

            for i in range(nT + 7):
                if live(i):
                    st_gather(i)
                if live(i - 2):
                    st_sd(i - 2)
                    st_ac(i - 2)
                if live(i - 3):
                    st_pe(i - 3)
                if live(i - 4):
                    st_at_mul(i - 4)
                if live(i - 5):
                    st_gbd(i - 5)
                if live(i - 6):
                    st_out(i - 6)
                if live(i - 7):
                    tiles.pop(i - 7, None)
            for p in reversed(score_ps):
                p.__exit__(None, None, None)

    nc.finalize()
    return nc


# ---------------------------------------------------------------- host
def _wrap_idx(flat_idx, kg):
    """[kg] int16 -> [128, kg//16] wrapped-16, replicated x8."""
    w = flat_idx.reshape(kg // 16, 16).T
    return np.tile(w, (8, 1))


def _rowbuf(nodes):
    r = nodes // NP
    i = nodes % NP
    in1 = i < H1N
    row = np.where(in1, H1N * r + i, H2N * r + (i - H1N))
    return row, in1


def kernel(X, edge_index, W1s, b1s, W2s, b2s, W1d, b1d, W2d, b2d, gamma):
    X = np.asarray(X)
    edge_index = np.asarray(edge_index)
    H = np.ascontiguousarray(X[:, -1]).astype(np.float32)          # (B, N, C)
    Hp = np.zeros((B, NPAD, C), np.float32)
    Hp[:, :N] = H

    I = edge_index[0].astype(np.int64)
    J = edge_index[1].astype(np.int64)
    core = J // NP

    percore = []
    nT1s, nT2s = [], []
    for c in range(NCORES):
        sel = np.where(core == c)[0]
        rI, b1I = _rowbuf(I[sel])
        Jloc = J[sel] - c * NP
        grp = np.where(b1I, 0, 1)
        order = np.lexsort((Jloc, grp))
        percore.append((sel[order], rI[order], Jloc[order], grp[order]))
        nT1s.append(int((grp == 0).sum()))
        nT2s.append(int((grp == 1).sum()))
    nT1 = -(-max(nT1s) // TILE)
    nT2 = -(-max(nT2s) // TILE)
    if (nT1 + nT2) % 2:
        nT2 += 1
    nT = nT1 + nT2

    key = (nT1, nT2)
    if key not in _PROGRAM_CACHE:
        _PROGRAM_CACHE[key] = build_program(nT1, nT2)
    nc = _PROGRAM_CACHE[key]

    # shared weight tensors
    W1 = np.stack([W1s, W1d]).astype(FP16)                        # (2, C, HID)
    B1 = np.stack([b1s, b1d]).astype(np.float32)[:, :, None]      # (2, HID, 1)
    W2 = np.stack([W2s, W2d]).astype(FP16)                        # (2, HID, R)
    B2 = np.stack(
        [np.tile(b2s[None, :], (128, 1)), np.tile(b2d[None, :], (128, 1))]
    ).astype(np.float32)                                          # (2, 128, R)

    gbd = np.zeros((128, 96), np.float32)
    gT = np.asarray(gamma, np.float32).T                          # (R, L)
    for g in range(4):
        for b in range(B):
            gbd[32 * g + 16 * b:32 * g + 16 * b + 16,
                24 * g + 12 * b:24 * g + 12 * b + 12] = gT
    GBDh = gbd.astype(FP16)

    iota = np.tile(np.arange(TILE, dtype=np.float64), (128, 1)).astype(FP16)

    in_maps = []
    unperm = []
    for c in range(NCORES):
        sel_s, rI_s, Jl_s, grp_s = percore[c]
        cnt1 = int((grp_s == 0).sum())
        cnt2 = len(sel_s) - cnt1
        idxA = np.zeros((nT, 128, 64), np.int16)
        idxW = np.zeros((nT, 128, 8), np.int16)
        idxW2 = np.zeros((nT, 128, 8), np.int16)
        thl = np.full((nT, 128), TILE, np.float32)
        pad_pos = np.full(nT * TILE, -1, np.int64)
        for gidx, base_t, cnt, off in ((0, 0, cnt1, 0), (1, nT1, cnt2, cnt1)):
            ntile_g = nT1 if gidx == 0 else nT2
            rI_g = rI_s[off:off + cnt]
            Jl_g = Jl_s[off:off + cnt]
            sel_g = sel_s[off:off + cnt]
            for t in range(ntile_g):
                e0 = t * TILE
                e1 = min(e0 + TILE, cnt)
                k = e1 - e0
                tt = base_t + t
                if k <= 0:
                    idxW[tt] = _wrap_idx(np.arange(128, dtype=np.int16), 128)
                    idxW2[tt] = _wrap_idx(np.arange(128, dtype=np.int16), 128)
                    continue
                ji = Jl_g[e0:e1]
                w0 = int(min(ji.min(), NP - 128))
                u = ji - w0
                assert u.max() < 128, (c, tt, int(u.max()))
                upad = np.concatenate([u, np.full(TILE - k, 127, np.int64)])
                ia = np.zeros(TILE, np.int64)
                ia[:k] = rI_g[e0:e1]
                idxA[tt] = _wrap_idx(ia.astype(np.int16), TILE)
                idxW[tt] = _wrap_idx(
                    (w0 + np.arange(128)).astype(np.int16), 128)
                w2 = w0 - 1 + np.arange(128)
                w2[0] = NP
                idxW2[tt] = _wrap_idx(w2.astype(np.int16), 128)
                thl[tt] = np.searchsorted(upad, np.arange(128), "left")
                pad_pos[tt * TILE: tt * TILE + k] = sel_g[e0:e1]
        unperm.append(pad_pos)

        HTs = np.ascontiguousarray(
            Hp[:, c * NP:(c + 1) * NP, :].transpose(0, 2, 1)
        ).astype(FP16)                                            # (B, C, NP)
        in_maps.append({
            "HT": HTs, "W1": W1, "B1": B1, "W2": W2, "B2": B2,
            "GBD": GBDh, "IOTA": iota, "IDXA": idxA, "IDXW": idxW,
            "IDXW2": idxW2,
            "THL": np.ascontiguousarray(thl.T),
        })

    import os
    import tempfile
    trace = bool(os.environ.get("BASS_KERNEL_TRACE"))
    tdir = None
    if trace:
        base = "/root/problem/work"
        tdir = tempfile.mkdtemp(prefix="ktrace_", dir=base if os.path.isdir(base) else None)
    res = run_bass_kernel_spmd(
        nc, in_maps, list(range(NCORES)), trace=trace, tmpdir=tdir,
    )
    if trace:
        kernel.last_trace_dir = tdir
        kernel.last_exec_time_ns = res.exec_time_ns

    logits = np.empty((B, L, E), np.float32)
    for c in range(NCORES):
        dev = res.results[c]["OUT"]                               # (96, nTP*512)
        # psum row p = 24*g4 + 12*b + l ; col = 512*P + 256*j + 128*c2 + e
        # edge slot = (2P+j)*1024 + 512*c2 + 128*g4 + e
        dv = np.asarray(dev).reshape(4, 2, L, nT // 2, 2, 256)
        dv = dv.transpose(1, 2, 3, 4, 0, 5).reshape(B, L, nT * TILE)
        pad_pos = unperm[c]
        valid = pad_pos >= 0
        logits[:, :, pad_pos[valid]] = dv[:, :, valid]
    return logits
